# revision 16
# baseline (speedup 1.0000x reference)
"""GQA forward kernel for 8 Trainium2 NeuronCores — fused/pipelined version.

Problem: B=2, T=2048, C=2048, 32 Q heads / 8 KV heads, head_dim=64, causal.

Sharding: 2-way data parallel over batch x 4-way tensor parallel over KV-head
pairs. Each core handles one batch element and 2 KV heads (8 Q heads), computes
its slice of Q/K/V projections, causal attention, and a partial output
projection (transposed). Host sums the 4 partials per batch and adds bo.

Structure:
  - Single fused loop over the 4 time-slices: project slice ts, then attend
    q-slice qs=ts (its K/V prefix 0..ts is complete), then out-project.
  - PSUM: proj(1 bank x2) + st(2 banks x2, shared with oproj) + av(2 banks) = 8.
  - Softmax denominators via the ones-column appended to V.
  - Few big DMAs (the ~1us SWDGE fixed cost per dma_start dominates issue).

Build flags (module-level, read by _build and _make_in_maps):
  USE_BF16: bf16 for all SBUF/DRAM tensors (else float32r like v1)
  MERGED_EXP: one [128,1024] exp across the two KV heads' score banks
"""

import sys
import numpy as np

T = 2048
C = 2048
D = 64
NT = 512          # q/t slice width (matmul moving free dim)
TS = T // NT      # 4 slices
KT = C // 128     # 16 contraction tiles
QC = 4            # local q-col tiles of 128 (512 local q cols)

USE_BF16 = True
MERGED_EXP = True

_CACHE = {}


def _ensure_path():
    for p in ("/opt/trn_rl_repo",):
        if p not in sys.path:
            sys.path.insert(0, p)


def _build(reps=1, sections=None):
    key = (reps, USE_BF16, MERGED_EXP, sections)
    if key in _CACHE:
        return _CACHE[key]
    _ensure_path()
    import concourse.mybir as mybir
    import concourse.bacc as bacc
    from concourse import tile
    from concourse.masks import make_identity
    from contextlib import ExitStack

    dt = mybir.dt
    f32 = dt.float32
    DT = dt.bfloat16 if USE_BF16 else dt.float32r
    AF = mybir.ActivationFunctionType

    nc = bacc.Bacc(None, target_bir_lowering=False)
    xt_d = nc.declare_dram_parameter("xt", (C, T), DT, isOutput=False)
    wq_d = nc.declare_dram_parameter("wq", (C, 512), DT, isOutput=False)
    wk_d = nc.declare_dram_parameter("wk", (C, 128), DT, isOutput=False)
    wv_d = nc.declare_dram_parameter("wv", (C, 128), DT, isOutput=False)
    wo_d = nc.declare_dram_parameter("wo", (512, C), DT, isOutput=False)
    bq_d = nc.declare_dram_parameter("bq", (512, 1), f32, isOutput=False)
    bk_d = nc.declare_dram_parameter("bk", (128, 1), f32, isOutput=False)
    bv_d = nc.declare_dram_parameter("bv", (128, 1), f32, isOutput=False)
    tri_d = nc.declare_dram_parameter("tri", (128, 128), DT, isOutput=False)
    opt_dt = dt.bfloat16 if USE_BF16 else f32
    opt_d = nc.declare_dram_parameter("opt", (C, T), opt_dt, isOutput=True)

    with tile.TileContext(nc) as tc, ExitStack() as ctx:
        constp = ctx.enter_context(tc.tile_pool(name="const", bufs=1))
        wp = ctx.enter_context(tc.tile_pool(name="w", bufs=1))
        pers = ctx.enter_context(tc.tile_pool(name="pers", bufs=1))
        xtp = ctx.enter_context(tc.tile_pool(name="xt", bufs=1))
        tmpp = ctx.enter_context(tc.tile_pool(name="tmp", bufs=2))
        ptp = ctx.enter_context(tc.tile_pool(name="pt", bufs=3 if USE_BF16 else 2))
        nb = 2 if USE_BF16 else 1  # f32r doesn't fit SBUF at bufs=2
        avsp = ctx.enter_context(tc.tile_pool(name="avs", bufs=nb))
        divp = ctx.enter_context(tc.tile_pool(name="div", bufs=2))
        otp = ctx.enter_context(tc.tile_pool(name="ot", bufs=nb))
        osp = ctx.enter_context(tc.tile_pool(name="os", bufs=nb))
        # PSUM: proj(1 bank x2) + st(2 banks x2, shared w/ oproj) + av(2) = 8
        pp_proj = ctx.enter_context(tc.tile_pool(name="pproj", bufs=2, space="PSUM"))
        pp_st = ctx.enter_context(tc.tile_pool(name="pst", bufs=2, space="PSUM"))
        pp_av = ctx.enter_context(tc.tile_pool(name="pav", bufs=1, space="PSUM"))

        # ---- constants & weights (loaded once, outside the reps loop)
        bq_s = []
        for j in range(QC):
            t = constp.tile([128, 1], f32, tag=f"bq{j}", name=f"bq{j}")
            nc.sync.dma_start(t[:], bq_d[j * 128:(j + 1) * 128, :])
            bq_s.append(t)
        bk_s = constp.tile([128, 1], f32, tag="bk", name="bk")
        nc.sync.dma_start(bk_s[:], bk_d[:])
        bv_s = constp.tile([128, 1], f32, tag="bv", name="bv")
        nc.sync.dma_start(bv_s[:], bv_d[:])
        tri_s = constp.tile([128, 128], DT, tag="tri", name="tri")
        nc.sync.dma_start(tri_s[:], tri_d[:])
        ident = constp.tile([128, 128], DT, tag="ident", name="ident")
        make_identity(nc, ident[:])
        zeros_c = constp.tile([128, 384], DT, tag="zeros_c", name="zeros_c")
        nc.vector.memset(zeros_c[:], 0.0)

        wq_s = [wp.tile([128, 512], DT, tag=f"wq{k}", name=f"wq{k}") for k in range(KT)]
        wk_s = [wp.tile([128, 128], DT, tag=f"wk{k}", name=f"wk{k}") for k in range(KT)]
        wv_s = [wp.tile([128, 128], DT, tag=f"wv{k}", name=f"wv{k}") for k in range(KT)]
        wo_s = [wp.tile([128, T], DT, tag=f"wo{h}", name=f"wo{h}") for h in range(4)]
        _loaded = set()

        def ensure_qkv_w(k):
            if ("qkv", k) in _loaded:
                return
            _loaded.add(("qkv", k))
            nc.sync.dma_start(wq_s[k][:], wq_d[k * 128:(k + 1) * 128, :])
            nc.sync.dma_start(wk_s[k][:], wk_d[k * 128:(k + 1) * 128, :])
            nc.sync.dma_start(wv_s[k][:], wv_d[k * 128:(k + 1) * 128, :])

        def ensure_wo():
            if "wo" in _loaded:
                return
            _loaded.add("wo")
            for h in range(4):
                nc.sync.dma_start(wo_s[h][:], wo_d[h * 128:(h + 1) * 128, :])

        qt_s = [pers.tile([128, T], DT, tag=f"qt{j}", name=f"qt{j}") for j in range(QC)]
        kt_s = pers.tile([128, T], DT, tag="kt", name="kt")
        v_s = [pers.tile([128, 130], DT, tag=f"vs{k}", name=f"vs{k}") for k in range(KT)]
        # ones columns (softmax denominator) are static: set once
        for k in range(KT):
            nc.vector.memset(v_s[k][:, 64:65], 1.0)
            nc.vector.memset(v_s[k][:, 129:130], 1.0)

        if reps != 1:
            for k in range(KT):
                ensure_qkv_w(k)
            ensure_wo()

        def body(_iv=None):
            # bf16: xt in two half-row groups (few big DMAs, fully resident).
            # f32: that would blow SBUF; per-slice tiles instead.
            if USE_BF16:
                xt_half = [[None] * KT, [None] * KT]

                def get_xt(ts):
                    g = ts // 2
                    if xt_half[g][0] is None:
                        for kt in range(KT):
                            ensure_qkv_w(kt)
                            t = xtp.tile(
                                [128, 2 * NT], DT,
                                tag=f"xt{g}_{kt}", name=f"xt{g}_{kt}",
                            )
                            nc.sync.dma_start(
                                t[:],
                                xt_d[kt * 128:(kt + 1) * 128,
                                     g * 2 * NT:(g + 1) * 2 * NT],
                            )
                            xt_half[g][kt] = t
                    off = (ts % 2) * NT
                    return [t[:, off:off + NT] for t in xt_half[g]]
            else:

                def get_xt(ts):
                    out = []
                    for kt in range(KT):
                        ensure_qkv_w(kt)
                        t = xtp.tile([128, NT], DT, tag=f"xts{kt}", name=f"xts{kt}")
                        nc.sync.dma_start(
                            t[:],
                            xt_d[kt * 128:(kt + 1) * 128, ts * NT:(ts + 1) * NT],
                        )
                        out.append(t[:])
                    return out

            def make_proj_units(ts2, xts2):
                # projection work for slice ts2 as 3 closures, to be emitted
                # interleaved between attention j-blocks of slice ts2-1 so the
                # PE has independent matmuls next to the exp-dependent ones
                lo2, hi2 = ts2 * NT, (ts2 + 1) * NT

                def proj_pair(specs):
                    # column tiles paired so consecutive matmuls alternate
                    # between the two proj psum banks
                    pss = [
                        pp_proj.tile([128, NT], f32, tag="proj", name="ps")
                        for _ in specs
                    ]
                    for kt in range(KT):
                        for (lhsT_fn, _), ps in zip(specs, pss):
                            nc.tensor.matmul(
                                ps[:], lhsT_fn(kt), xts2[kt],
                                start=(kt == 0), stop=(kt == KT - 1),
                            )
                    for (_, wout), ps in zip(specs, pss):
                        wout(ps)

                def unit_q(j0):
                    def run():
                        proj_pair([
                            (
                                (lambda kt, j=j: wq_s[kt][:, j * 128:(j + 1) * 128]),
                                (lambda ps, j=j: nc.vector.tensor_scalar_add(
                                    qt_s[j][:, lo2:hi2], ps[:], bq_s[j][:])),
                            )
                            for j in (j0, j0 + 1)
                        ])
                    return run

                def unit_kv():
                    def run():
                        vtmp = tmpp.tile([128, NT], DT, tag="vtmp", name="vtmp")
                        proj_pair([
                            (
                                (lambda kt: wk_s[kt][:]),
                                (lambda ps: nc.vector.tensor_scalar_add(
                                    kt_s[:, lo2:hi2], ps[:], bk_s[:])),
                            ),
                            (
                                (lambda kt: wv_s[kt][:]),
                                (lambda ps: nc.vector.tensor_scalar_add(
                                    vtmp[:], ps[:], bv_s[:])),
                            ),
                        ])
                        # transpose V slice into kpos-major v_s tiles
                        for r in range(4):
                            k4 = ts2 * 4 + r
                            tp_ps = pp_proj.tile(
                                [128, 128], DT, tag="proj", name="tp"
                            )
                            nc.tensor.transpose(
                                tp_ps[:], vtmp[:, r * 128:(r + 1) * 128], ident[:]
                            )
                            nc.vector.tensor_copy(v_s[k4][:, 0:64], tp_ps[:, 0:64])
                            nc.vector.tensor_copy(
                                v_s[k4][:, 65:129], tp_ps[:, 64:128]
                            )
                    return run

                return [unit_q(0), unit_q(2), unit_kv()]

            # prologue: slice-0 projections run unpipelined
            for u in make_proj_units(0, get_xt(0)):
                u()
            ensure_wo()

            for ts in range(TS):
                qs = ts
                lo, hi = ts * NT, (ts + 1) * NT
                if ts + 2 < TS:
                    get_xt(ts + 2)  # issue the next half-row group's DMAs early
                units = (
                    make_proj_units(ts + 1, get_xt(ts + 1))
                    if ts + 1 < TS else []
                )

                # ---- attention for q-slice qs (= ts)
                nkt = 4 * qs + 4
                ots = []
                for j in range(QC):
                    av = pp_av.tile([128, 2 * NT], f32, tag="av", name="av")
                    for kt in range(nkt):
                        stp = pp_st.tile([128, 2 * NT], f32, tag="st", name="st")
                        nc.tensor.matmul(
                            stp[:, 0:NT],
                            kt_s[0:64, kt * 128:(kt + 1) * 128],
                            qt_s[j][0:64, lo:hi],
                            start=True, stop=True,
                        )
                        nc.tensor.matmul(
                            stp[:, NT:2 * NT],
                            kt_s[64:128, kt * 128:(kt + 1) * 128],
                            qt_s[j][64:128, lo:hi],
                            start=True, stop=True,
                        )
                        pt = ptp.tile([128, 2 * NT], DT, tag="pt", name="pt")
                        r = kt - 4 * qs
                        if r < 0:
                            if MERGED_EXP:
                                nc.scalar.activation(
                                    pt[:], stp[:], AF.Exp, scale=0.125
                                )
                            else:
                                nc.scalar.activation(
                                    pt[:, 0:NT], stp[:, 0:NT], AF.Exp, scale=0.125
                                )
                                nc.scalar.activation(
                                    pt[:, NT:2 * NT], stp[:, NT:2 * NT],
                                    AF.Exp, scale=0.125,
                                )
                        else:
                            if MERGED_EXP:
                                nc.scalar.activation(
                                    pt[:, r * 128:2 * NT],
                                    stp[:, r * 128:2 * NT],
                                    AF.Exp, scale=0.125,
                                )
                            else:
                                nc.scalar.activation(
                                    pt[:, r * 128:NT], stp[:, r * 128:NT],
                                    AF.Exp, scale=0.125,
                                )
                                nc.scalar.activation(
                                    pt[:, NT + r * 128:2 * NT],
                                    stp[:, NT + r * 128:2 * NT],
                                    AF.Exp, scale=0.125,
                                )
                            if r > 0:
                                nc.vector.tensor_copy(
                                    pt[:, 0:r * 128], zeros_c[:, 0:r * 128]
                                )
                                nc.vector.tensor_copy(
                                    pt[:, NT:NT + r * 128], zeros_c[:, 0:r * 128]
                                )
                            nc.vector.tensor_mul(
                                pt[:, r * 128:(r + 1) * 128],
                                pt[:, r * 128:(r + 1) * 128],
                                tri_s[:],
                            )
                            nc.vector.tensor_mul(
                                pt[:, NT + r * 128:NT + (r + 1) * 128],
                                pt[:, NT + r * 128:NT + (r + 1) * 128],
                                tri_s[:],
                            )
                        nc.tensor.matmul(
                            av[0:65, 0:NT], v_s[kt][:, 0:65], pt[:, 0:NT],
                            start=(kt == 0), stop=(kt == nkt - 1),
                        )
                        nc.tensor.matmul(
                            av[0:65, NT:2 * NT], v_s[kt][:, 65:130],
                            pt[:, NT:2 * NT],
                            start=(kt == 0), stop=(kt == nkt - 1),
                        )
                    # move av out of PSUM quickly, then divide by the denom row
                    avs = avsp.tile([65, 2 * NT], f32, tag="avs", name="avs")
                    nc.vector.tensor_copy(avs[:], av[0:65, :])
                    rc = divp.tile([1, 2 * NT], f32, tag="rc", name="rc")
                    nc.vector.reciprocal(rc[:], avs[64:65, :])
                    bc = divp.tile([64, 2 * NT], f32, tag="bc", name="bc")
                    nc.gpsimd.partition_broadcast(bc[:], rc[:])
                    ot_j = otp.tile([128, NT], DT, tag=f"ot{j}", name=f"ot{j}")
                    nc.vector.tensor_mul(ot_j[0:64, :], avs[0:64, 0:NT], bc[:, 0:NT])
                    nc.vector.tensor_mul(
                        ot_j[64:128, :], avs[0:64, NT:2 * NT], bc[:, NT:2 * NT]
                    )
                    ots.append(ot_j)
                    if units:
                        units.pop(0)()

                while units:
                    units.pop(0)()

                # ---- output projection for this q-slice (shares st psum
                # slots); store DMAs issue from the idle Pool queue so their
                # ~1us SWDGE fixed cost stays off the SP sequencer
                for ct in range(KT):
                    op_ps = pp_st.tile([128, NT], f32, tag="st", name="op")
                    for h in range(4):
                        nc.tensor.matmul(
                            op_ps[:],
                            wo_s[h][:, ct * 128:(ct + 1) * 128],
                            ots[h][:],
                            start=(h == 0), stop=(h == 3),
                        )
                    os_t = osp.tile([128, NT], opt_dt, tag="os", name="os")
                    nc.vector.tensor_copy(os_t[:], op_ps[:])
                    # scalar engine issues on the qActDynamicHW HWDGE ring:
                    # keeps descriptor-gen off SP and off the SWDGE/Pool path
                    nc.scalar.dma_start(
                        opt_d[ct * 128:(ct + 1) * 128, lo:hi], os_t[:]
                    )

        if sections == "static":
            for _ in range(reps):
                body()
        elif reps == 1:
            body()
        else:
            with tc.For_i(0, reps, 1) as _i:
                body(_i)

    nc.compile()
    _CACHE[key] = nc
    return nc


def _make_in_maps(inputs):
    if USE_BF16:
        import ml_dtypes

        cdt = ml_dtypes.bfloat16
    else:
        cdt = np.float32
    x = np.asarray(inputs["x"], np.float32)
    Wq = np.asarray(inputs["Wq"], np.float32)
    bq = np.asarray(inputs["bq"], np.float32)
    Wk = np.asarray(inputs["Wk"], np.float32)
    bk = np.asarray(inputs["bk"], np.float32)
    Wv = np.asarray(inputs["Wv"], np.float32)
    bv = np.asarray(inputs["bv"], np.float32)
    Wo = np.asarray(inputs["Wo"], np.float32)

    tri = np.triu(np.ones((128, 128), np.float32)).astype(cdt)
    in_maps = []
    for c in range(8):
        b, tp = c // 4, c % 4
        k0, k1 = 2 * tp, 2 * tp + 1
        qorder = np.concatenate(
            [
                np.r_[(4 * k + j) * D:(4 * k + j + 1) * D]
                for j in range(4)
                for k in (k0, k1)
            ]
        )
        kvorder = np.r_[k0 * D:(k0 + 1) * D, k1 * D:(k1 + 1) * D]
        in_maps.append(
            {
                "xt": np.ascontiguousarray(x[b].T.astype(cdt)),
                "wq": np.ascontiguousarray(Wq[:, qorder].astype(cdt)),
                "wk": np.ascontiguousarray(Wk[:, kvorder].astype(cdt)),
                "wv": np.ascontiguousarray(Wv[:, kvorder].astype(cdt)),
                "wo": np.ascontiguousarray(Wo[qorder, :].astype(cdt)),
                "bq": np.ascontiguousarray(bq[qorder].reshape(512, 1)),
                "bk": np.ascontiguousarray(bk[kvorder].reshape(128, 1)),
                "bv": np.ascontiguousarray(bv[kvorder].reshape(128, 1)),
                "tri": tri,
            }
        )
    return in_maps


def _gather(results, bo):
    out = np.zeros((2, T, C), np.float32)
    for c in range(8):
        out[c // 4] += np.asarray(results[c]["opt"], np.float32).T
    out += bo.astype(np.float32)
    return out


def kernel(**inputs):
    _ensure_path()
    from concourse.bass_utils import run_bass_kernel_spmd

    nc = _build(reps=1)
    in_maps = _make_in_maps(inputs)
    res = run_bass_kernel_spmd(nc, in_maps, list(range(8)))
    return _gather(res.results, np.asarray(inputs["bo"], np.float32))


def run_timed(inputs, reps, n_calls=3, sections=None):
    """Wall-clock the SPMD call at a given in-kernel rep count; returns
    (best_wall_seconds, outputs). Kernel time per rep is isolated by
    differencing two rep counts (data transfer is identical)."""
    import time

    _ensure_path()
    from concourse.bass_utils import run_bass_kernel_spmd

    nc = _build(reps=reps)
    in_maps = _make_in_maps(inputs)
    best = None
    res = None
    for _ in range(n_calls):
        t0 = time.time()
        res = run_bass_kernel_spmd(nc, in_maps, list(range(8)))
        dtm = time.time() - t0
        best = dtm if best is None else min(best, dtm)
    return best, _gather(res.results, np.asarray(inputs["bo"], np.float32))
